# revision 38
# baseline (speedup 1.0000x reference)
"""LocallyGroupedAttn (windowed attention, ws=7, 8 heads) on 8 trn2 cores.

Sharding: data-parallel over batch B=8, one image per NeuronCore.

Host-side prep (part of sharding/layout): x is cast to bf16 and laid out as
x^T slabs [slab, chalf, c(128), tau(1024)] where tokens use pair-major order
    tau = 128*a + 64*wi + 7*k + j   (pair a, parity wi, col-in-window k,
                                     row j; 98 of 128 pair columns used)
so every window is a contiguous 49-column slice (matmul stationary operands
must have 1-D free access patterns), qkv matmul streams stay unpermuted, and
the 128 pair pitch lets whole pairs ([0:113] slices incl. 15 garbage columns)
feed single merged matmuls whose output rows land at the legal {0, 64}
partition bases. Weights are pre-transposed/cast on host; biases
pre-broadcast; a 128x128 identity is shipped for PE-transposes.

Per-core pipeline per slab (16 slabs of 784 real tokens):
  load xT [128, 2, 1024]
  q^T,k^T = wT-stationary matmuls (N=512)      -> ACT drain (+bias, bf16)
  v       = one pair-wide matmul per pair      -> DVE drain (+bias, bf16)
  scores^T[tk,tq] per (window,head), 4-head row-strip packing on PE
  exp on ACT (softmax scale folded into activation scale), bf16
  o_un[tq,d] + denom: PV matmul with ones-augmented v
  o = o_un * recip(denom) (DVE free-broadcast)  -> bf16
  o --PE identity transpose--> o^T (ACT psum drain, bf16)
  proj: one matmul per chalf over the whole pair ; +bias (DVE) ; DMA out.
Window pairs sit at partition strips {0:49, 64:113} in every per-pair tile
(64 is a legal matmul output base partition; 49 is not).
"""

import json
import os

import numpy as np
import ml_dtypes

import concourse.bass as bass
import concourse.bass2jax as bass2jax
import concourse.tile as tile
from concourse import mybir
from concourse.bass_utils import compile_bir_kernel as _real_compile_bir_kernel
from concourse.bass_utils import run_bass_kernel_spmd


def _split_multi_waits(bir_bytes):
    """This container's walrus accepts at most ONE sync wait per instruction
    ("Too many sync wait commands"). Split extra waits onto standalone
    same-engine EventSemaphore wait carriers placed just before."""
    m = json.loads(bir_bytes)
    ctr = 0
    for f in m["functions"]:
        for blk in f["blocks"]:
            out = []
            for ins in blk.get("instructions", []):
                si = ins.get("sync_info")
                if si:
                    waits = si.get("on_wait") or []
                    if len(waits) > 1:
                        for wt in waits[:-1]:
                            ctr += 1
                            out.append({
                                "debug": ins.get("debug", 0),
                                "engine": ins["engine"],
                                "ins": [],
                                "outs": [],
                                "name": f"WSPLIT-{ctr}",
                                "opcode": "EventSemaphore",
                                "sync_info": {"on_update": [], "on_wait": [wt]},
                            })
                        si["on_wait"] = [waits[-1]]
                out.append(ins)
            blk["instructions"] = out
    return json.dumps(m).encode()


def _patched_compile_bir_kernel(bir_json, tmpdir, neff_name="file.neff"):
    if isinstance(bir_json, str):
        bir_json = bir_json.encode()
    return _real_compile_bir_kernel(_split_multi_waits(bir_json), tmpdir, neff_name)


bass2jax.compile_bir_kernel = _patched_compile_bir_kernel

F32 = mybir.dt.float32
F16 = mybir.dt.float16
BF16 = mybir.dt.bfloat16
AF = mybir.ActivationFunctionType
OP = mybir.AluOpType

B, H, W, C = 8, 112, 112, 256
WS, NH, HD = 7, 8, 32
N = H * W                     # 12544 tokens per image
SLAB_T = WS * W               # 784 real tokens per slab
NSLAB = H // WS               # 16
NWIN_ROW = W // WS            # 16 windows per slab
NPAIR = NWIN_ROW // 2         # 8 pairs per slab
WS2 = WS * WS                 # 49
PP = 128                      # pair pitch: windows at tau {0:49, 64:113}
PADT = PP * NPAIR             # 1024 padded tau columns per slab
SCALE = float(HD) ** -0.5


def build_bass() -> bass.Bass:
    nslab = int(os.environ.get("KBUILD_SLABS", NSLAB))
    ncouple = int(os.environ.get("KBUILD_COUPLES", NPAIR // 2))
    stage = int(os.environ.get("KBUILD_STAGE", "4"))
    repeat = int(os.environ.get("KBUILD_REPEAT", "1"))
    ablate = set(os.environ.get("KBUILD_ABLATE", "").split(","))
    # Cumulative stage keep-level: 0=DMA only, 1=+qkv, 2=+v, 3=+scores,
    # 4=+exp, 5=+pv, 6=+dve(norm), 7=+transpose, 8=full(proj+ob).
    only = int(os.environ.get("KBUILD_ONLY", "8"))
    lvl = {
        "qkv": 1, "v": 2, "vb": 2, "scores": 3, "exp": 4, "pv": 5,
        "dve": 6, "tp": 7, "otc": 7, "proj": 8, "ob": 8,
    }
    for k, v in lvl.items():
        if v > only:
            ablate.add(k)
    nc = bass.Bass()
    xT_in = nc.dram_tensor("xT", [NSLAB, 2, 128, PADT], BF16, kind="ExternalInput")
    wT_in = nc.dram_tensor("wT", [2, 128, 3 * C], BF16, kind="ExternalInput")
    wpT_in = nc.dram_tensor("wpT", [2, 128, C], BF16, kind="ExternalInput")
    cb_in = nc.dram_tensor("cb", [128, 4 + C], F32, kind="ExternalInput")
    id_in = nc.dram_tensor("ident", [128, 128], BF16, kind="ExternalInput")
    # 120 partition rows (not 113): SDMA splits DMAs across engines in
    # 8-partition groups; non-multiple-of-8 partition counts fall off a
    # cliff (measured ~21 GB/s vs ~80+ GB/s). Rows 113:120 are zero filler.
    out = nc.dram_tensor("out", [NSLAB, 120, NPAIR, C], F16, kind="ExternalOutput")

    with tile.TileContext(nc) as tc:
        with (
            tc.tile_pool(name="consts", bufs=1) as consts,
            tc.tile_pool(name="xt", bufs=3) as xt_pool,
            tc.tile_pool(name="qk", bufs=3) as qk_pool,
            tc.tile_pool(name="vt", bufs=4) as vt_pool,
            tc.tile_pool(name="es", bufs=6) as es_pool,
            tc.tile_pool(name="dn", bufs=6) as dn_pool,
            tc.tile_pool(name="ob", bufs=6) as o_pool,
            tc.tile_pool(name="ot", bufs=6) as ot_pool,
            tc.tile_pool(name="os", bufs=4) as out_pool,
            tc.tile_pool(name="ps", bufs=1, space="PSUM") as psum,
        ):
            # ---- constants -------------------------------------------------
            wT = consts.tile([128, 2, 3 * C], BF16)
            for ch in range(2):
                nc.gpsimd.dma_start(out=wT[:, ch, :], in_=wT_in[ch])
            wpT = consts.tile([128, 2, C], BF16)
            for ch in range(2):
                nc.gpsimd.dma_start(out=wpT[:, ch, :], in_=wpT_in[ch])
            cb = consts.tile([128, 4 + C], F32)
            nc.gpsimd.dma_start(out=cb, in_=cb_in[:, :])
            ident = consts.tile([128, 128], BF16)
            nc.gpsimd.dma_start(out=ident, in_=id_in[:, :])
            qkb = cb[:, 0:4]
            # pb includes the folded v-bias: pb2 = proj_b + qkv_b[2C:] @ proj_w.T
            # (softmax rows sum to 1, so +b_v on v adds exactly b_v to o_norm)
            pb = cb[:, 4 : 4 + C]
            vts = []
            for i in range(4):
                vt_p = consts.tile([128, NH, HD + 1], BF16, name=f"vtp{i}")
                nc.gpsimd.memset(vt_p[:, :, HD : HD + 1], 1.0)
                vts.append(vt_p)

            # ---- main loop: software-pipelined across couples ----------
            # stage A(c): xT/qk loads + v-projs + qk^T matmuls + exp
            # stage B(c): PV + recip + o-norm + o^T transposes   (lag 1)
            # stage C(c): proj matmuls + drain into slab staging (lag 2)
            slab_state = {}

            def stage_a(gc):
                r, cp = divmod(gc, NPAIR // 2)
                if cp == 0:
                    xT = xt_pool.tile([128, 2, PADT], BF16, name=f"xT_{r}", tag="xt")
                    for ch in range(2 if "dmain" not in ablate else 0):
                        nc.sync.dma_start(out=xT[:, ch, :], in_=xT_in[r, ch])
                    # stream x^T with the dead tail of each pair (cols
                    # 113:128) sliced away: N=452 per half instead of 512.
                    qk_sb = qk_pool.tile([128, 4, 904], BF16, name=f"qk_{r}", tag="qk")
                    xTs = xT.rearrange("p c (a q) -> p c a q", q=PP)[:, :, :, 0:113]
                    for j in range(4 if "qkv" not in ablate else 0):
                        for half in range(2):
                            ps = psum.tile([128, 512], F32, tag="mm", bufs=2)
                            for ch in range(2):
                                nc.tensor.matmul(
                                    ps[:, 0:452],
                                    lhsT=wT[:, ch, 128 * j : 128 * (j + 1)],
                                    rhs=xTs[:, ch, 4 * half : 4 * (half + 1)],
                                    start=(ch == 0),
                                    stop=(ch == 1),
                                )
                            if j % 2 == 0:
                                nc.scalar.activation(
                                    out=qk_sb[:, j, 452 * half : 452 * (half + 1)],
                                    in_=ps[:, 0:452],
                                    func=AF.Identity,
                                    bias=qkb[:, j : j + 1],
                                    scale=1.0,
                                )
                            else:
                                nc.vector.tensor_tensor(
                                    qk_sb[:, j, 452 * half : 452 * (half + 1)],
                                    ps[:, 0:452],
                                    qkb[:, j : j + 1].to_broadcast([128, 452]),
                                    OP.add,
                                )
                    out_sb = out_pool.tile([128, NPAIR, C], F16, name=f"os_{r}", tag="os")
                    if "dmaout" in ablate:
                        pass
                    elif "ob" in ablate:
                        nc.gpsimd.memset(out_sb[:, :, :], 0.0)
                    else:
                        # engines need 32-aligned partition bases; rows
                        # 96:113 get overwritten by the real proj output
                        nc.gpsimd.memset(out_sb[96:128, :, :], 0.0)
                    slab_state[r] = (xT, qk_sb, out_sb)
                xT, qk_sb, out_sb = slab_state[r]

                vt_c = []
                for idx, a in enumerate((2 * cp, 2 * cp + 1)):
                    if "v" in ablate and "vb" in ablate:
                        vt_c.append(vts[(2 * cp + idx) % 4])
                        continue
                    ps_v = psum.tile([128, 512], F32, tag="mm", bufs=2)
                    for ch in range(0 if "v" in ablate else 2):
                        nc.tensor.matmul(
                            ps_v[0:113, 0:C],
                            lhsT=xT[:, ch, PP * a :][:, 0:113],
                            rhs=wT[:, ch, 2 * C : 3 * C],
                            start=(ch == 0),
                            stop=(ch == 1),
                        )
                    vt = vts[(2 * cp + idx) % 4]
                    if "vb" not in ablate:
                      # v-bias folded into pb on host; plain ACT drain/cast
                      nc.scalar.activation(
                        out=vt[0:113, :, 0:HD],
                        in_=ps_v[0:113, 0:C].rearrange("p (h d) -> p h d", h=NH),
                        func=AF.Identity,
                        scale=1.0,
                      )
                    vt_c.append(vt)

                if "scores" in ablate and "exp" in ablate:
                    return {"es": None, "vt_c": vt_c, "r": r, "cp": cp}
                # scores packed into 2 PSUM banks (chunk c at 512*(c//2) +
                # 196*(c%2)) so the pool can double-buffer: scores of couple
                # c+1 overlap the exp drain of couple c.
                SREPACK = os.environ.get("KBUILD_SREPACK", "0") == "1"
                if SREPACK:
                    ps_s = psum.tile([128, 1024], F32, tag="s", bufs=int(os.environ.get("KBUILD_SBUFS","2")))
                else:
                    ps_s = psum.tile([128, 2048], F32, tag="s", bufs=1)
                es = (
                    es_pool.tile([128, 4 * 196], BF16, name="es")
                    if "exp" not in ablate
                    else None
                )
                for idx, a in enumerate(
                    () if "scores" in ablate else (2 * cp, 2 * cp + 1)
                ):
                    for wi, b0 in ((0, 0), (1, 64)):
                        tau0 = 113 * a + 64 * wi
                        for h in range(NH):
                            j = h // 4
                            p0 = 32 * (h % 4)
                            c4 = h % 4
                            if SREPACK:
                                sc = (
                                    512 * (c4 // 2)
                                    + 196 * (c4 % 2)
                                    + 98 * idx
                                    + WS2 * (h // 4)
                                )
                            else:
                                sc = 512 * c4 + 98 * idx + WS2 * (h // 4)
                            nc.tensor.matmul(
                                ps_s[b0 : b0 + WS2, sc : sc + WS2],
                                lhsT=qk_sb[p0 : p0 + 32, 2 + j, tau0:][:, 0:WS2],
                                rhs=qk_sb[p0 : p0 + 32, j, tau0:][:, 0:WS2],
                                start=True,
                                stop=True,
                                tile_position=(p0, b0),
                            )
                if "exp" not in ablate:
                    if SREPACK:
                        nc.scalar.activation(
                            out=es[0:113, :].rearrange(
                                "p (a b x) -> p a b x", a=2, b=2
                            ),
                            in_=ps_s[0:113, :].rearrange("p (a y) -> p a y", a=2)[
                                :, :, 0:392
                            ].rearrange("p a (b x) -> p a b x", b=2)[:, :, :, 0:196],
                            func=AF.Exp,
                            scale=SCALE,
                        )
                    else:
                        nc.scalar.activation(
                            out=es[0:113, :].rearrange("p (s x) -> p s x", s=4),
                            in_=ps_s[0:113, :].rearrange("p (s x) -> p s x", s=4)[
                                :, :, 0:196
                            ],
                            func=AF.Exp,
                            scale=SCALE,
                        )
                return {"es": es, "vt_c": vt_c, "r": r, "cp": cp}

            def stage_b(ctx):
                es, vt_c = ctx["es"], ctx["vt_c"]
                if "pv" in ablate and "dve" in ablate:
                    ctx["o_sbs"] = None
                    return ctx
                ps_oe = psum.tile([128, NH * (HD + 1)], F32, tag="oe", bufs=1)
                ps_oo = psum.tile([128, NH * (HD + 1)], F32, tag="oo", bufs=1)
                ps_par = [ps_oe, ps_oo]
                for idx in range(0 if "pv" in ablate else 2):
                    r0 = 64 * idx
                    vt = vt_c[idx]
                    for par, e0 in ((0, 0), (1, 64)):
                        for h in range(NH):
                            ec = 196 * (h % 4) + 98 * idx + WS2 * (h // 4)
                            nc.tensor.matmul(
                                ps_par[par][r0 : r0 + WS2, 33 * h : 33 * (h + 1)],
                                lhsT=es[e0 : e0 + WS2, ec : ec + WS2],
                                rhs=vt[e0 : e0 + WS2, h, :],
                                start=True,
                                stop=True,
                            )
                if "dve" in ablate:
                    ctx["o_sbs"] = None
                    return ctx
                o_sbs = []
                for par in range(2):
                    ps_o = ps_par[par]
                    den = dn_pool.tile([128, NH], F32)
                    if "dve" not in ablate:
                      nc.vector.reciprocal(
                        den[0:113, :],
                        ps_o[0:113, :].rearrange("p (h e) -> p h e", h=NH)[
                            :, :, HD : HD + 1
                        ],
                      )
                    o_sb = o_pool.tile([128, C], BF16)
                    if "dve" not in ablate:
                      nc.vector.tensor_tensor(
                        o_sb[0:113, :].rearrange("p (h d) -> p h d", h=NH),
                        ps_o[0:113, :].rearrange("p (h e) -> p h e", h=NH)[
                            :, :, 0:HD
                        ],
                        den[0:113, :, None].to_broadcast([113, NH, HD]),
                        OP.mult,
                      )
                    o_sbs.append(o_sb)
                ctx["o_sbs"] = o_sbs
                return ctx

            def stage_b2(ctx):
                o_sbs = ctx["o_sbs"]
                if "tp" in ablate and "otc" in ablate:
                    ctx["oT"] = None
                    return ctx
                ps_t = psum.tile([128, 456], BF16, tag="oo", bufs=1)
                for par in range(2):
                    for ch in range(0 if "tp" in ablate else 2):
                        nc.tensor.transpose(
                            ps_t[:, 228 * par + 114 * ch :][:, 0:113],
                            in_=o_sbs[par][0:113, 128 * ch : 128 * (ch + 1)],
                            identity=ident[0:113, 0:113],
                        )
                oT = ot_pool.tile([128, 2, 2, 114], BF16)
                if "otc" not in ablate:
                  nc.scalar.activation(
                    out=oT[:, :, :, :].rearrange("p q c t -> p (q c t)"),
                    in_=ps_t[:, :],
                    func=AF.Identity,
                    scale=1.0,
                  )
                ctx["oT"] = oT
                return ctx

            def stage_c(ctx):
                r, cp, oT = ctx["r"], ctx["cp"], ctx["oT"]
                out_sb = slab_state[r][2]
                for par in range(2 if not ("proj" in ablate and "ob" in ablate) else 0):
                    ps_p = psum.tile([128, 512], F32, tag="mm", bufs=2)
                    for ch in range(0 if "proj" in ablate else 2):
                        nc.tensor.matmul(
                            ps_p[0:113, 0:C],
                            lhsT=oT[:, par, ch, 0:113],
                            rhs=wpT[:, ch, :],
                            start=(ch == 0),
                            stop=(ch == 1),
                        )
                    if "ob" not in ablate:
                      nc.vector.tensor_tensor(
                        out_sb[0:113, 2 * cp + par, :],
                        ps_p[0:113, 0:C],
                        pb[0:113, :],
                        OP.add,
                      )
                if cp == NPAIR // 2 - 1:
                    if "dmaout" not in ablate:
                        nc.sync.dma_start(out=out[r], in_=out_sb[0:120, :, :])
                    del slab_state[r]

            ncpl = NPAIR // 2
            total = nslab * ncpl
            grand = repeat * total
            lagb = int(os.environ.get("KBUILD_LAGB", "1"))
            lagb2 = int(os.environ.get("KBUILD_LAGB2", "3"))
            lagc = int(os.environ.get("KBUILD_LAGC", "5"))
            ctxs = {}
            for gc in range(grand + lagc):
                if gc < grand:
                    ctxs[gc] = stage_a(gc % total)
                if 0 <= gc - lagb < grand:
                    ctxs[gc - lagb] = stage_b(ctxs[gc - lagb])
                if 0 <= gc - lagb2 < grand:
                    ctxs[gc - lagb2] = stage_b2(ctxs[gc - lagb2])
                if gc - lagc >= 0:
                    stage_c(ctxs.pop(gc - lagc))
    return nc


def _unscramble(o_perm):
    """[16, 120, 8, 256] staging -> [N, C]. Rows {0:49, 64:113} of
    parity-tile (2cp+par) are windows 4cp+par / 4cp+2+par in m=7k+j order."""
    o = np.empty((NSLAB, NWIN_ROW, WS, WS, C), dtype=np.float32)  # [s,w,k,j,c]
    for cp in range(4):
        for par in range(2):
            pt = o_perm[:, :, 2 * cp + par]
            o[:, 4 * cp + par] = pt[:, 0:WS2].reshape(NSLAB, WS, WS, C)
            o[:, 4 * cp + 2 + par] = pt[:, 64:113].reshape(NSLAB, WS, WS, C)
    # [s, w, k, j, c] -> t = 112j + 7w + k within slab
    o = o.transpose(0, 3, 1, 2, 4)  # [s, j, w, k, c]
    return o.reshape(N, C)


def slab_out(out, t0, w):
    """HBM rows of window w in (k, j, c) order matching out_sb row order."""
    return out[t0 : t0 + SLAB_T, :].rearrange(
        "(j w k) c -> w k j c", j=WS, w=NWIN_ROW, k=WS
    )[w]


def _prep_host(x, qkv_w, qkv_b, proj_w, proj_b):
    """Host-side layout prep. Returns (xT [B,16,2,128,1024] bf16, shared)."""
    bf16 = ml_dtypes.bfloat16
    # x -> [B, slab, w, k, j, c] -> tau = 128a + 64wi + (7k + j) pair order
    xs = x.reshape(B, NSLAB, WS, NWIN_ROW, WS, C)
    xw = xs.transpose(0, 1, 3, 4, 2, 5).reshape(B, NSLAB, NWIN_ROW, WS2, C)
    xpad = np.zeros((B, NSLAB, NPAIR, PP, C), dtype=np.float32)
    xp = xw.reshape(B, NSLAB, NPAIR, 2, WS2, C)
    xpad[:, :, :, 0:WS2] = xp[:, :, :, 0]
    xpad[:, :, :, 64 : 64 + WS2] = xp[:, :, :, 1]
    # [B, slab, tau, c] -> [B, slab, ch, c(128), tau]
    xt = xpad.reshape(B, NSLAB, PADT, C).transpose(0, 1, 3, 2)
    xt = xt.reshape(B, NSLAB, 2, 128, PADT).astype(bf16)
    xt = np.ascontiguousarray(xt)

    wT = np.ascontiguousarray(
        qkv_w.T.reshape(2, 128, 3 * C).astype(bf16)
    )
    wpT = np.ascontiguousarray(proj_w.T.reshape(2, 128, C).astype(bf16))
    cb = np.empty((128, 4 + C), dtype=np.float32)
    cb[:, 0:4] = qkv_b[0 : 2 * C].reshape(4, 128).T
    # v-bias folded into the proj bias: softmax rows sum to 1, so adding
    # b_v to v adds exactly b_v to the normalized o, and proj is linear.
    pb2 = proj_b + qkv_b[2 * C :] @ proj_w.T
    cb[:, 4 : 4 + C] = np.broadcast_to(pb2, (128, C))
    ident = np.eye(128, dtype=bf16)
    return xt, {"wT": wT, "wpT": wpT, "cb": cb, "ident": ident}


_NC_CACHE = None


def _get_nc():
    global _NC_CACHE
    if _NC_CACHE is None:
        _NC_CACHE = build_bass()
    return _NC_CACHE


def kernel(x, qkv_w, qkv_b, proj_w, proj_b, H=None, W=None, **_ignored):
    x = np.ascontiguousarray(np.asarray(x, dtype=np.float32))
    assert x.shape == (B, N, C), x.shape
    xt, shared = _prep_host(
        x,
        np.asarray(qkv_w, dtype=np.float32),
        np.asarray(qkv_b, dtype=np.float32),
        np.asarray(proj_w, dtype=np.float32),
        np.asarray(proj_b, dtype=np.float32),
    )
    nc = _get_nc()
    in_maps = [{"xT": np.ascontiguousarray(xt[b]), **shared} for b in range(B)]
    res = run_bass_kernel_spmd(nc, in_maps, core_ids=list(range(B)))
    return np.stack(
        [_unscramble(r["out"]) for r in res.results], axis=0
    )


if __name__ == "__main__":
    rng = np.random.default_rng(0)
    inputs = {
        "x": rng.standard_normal((B, N, C), dtype=np.float32),
        "qkv_w": rng.standard_normal((3 * C, C), dtype=np.float32) / 16.0,
        "qkv_b": rng.standard_normal((3 * C,), dtype=np.float32) * 0.02,
        "proj_w": rng.standard_normal((C, C), dtype=np.float32) / 16.0,
        "proj_b": rng.standard_normal((C,), dtype=np.float32) * 0.02,
    }
    o = kernel(**inputs)
    print(o.shape, o.dtype, float(np.abs(o).mean()))

